# revision 6
# baseline (speedup 1.0000x reference)
"""Trainium2 Bass kernel for nn_DFlashAttention_43774306681111.

Full-attention transformer block: QKV projection + per-head RMSNorm + neox
RoPE + GQA softmax attention (non-causal) + output projection.

Sharding (8 cores): 2-way data parallel over batch x 4-way tensor parallel
over heads. Core c handles batch c//4 and head group c%4 (q heads
4g..4g+3, kv head g). Each core computes a partial output [S, HID]
(its heads' contribution through Wo); the host sums the 4 partials per
batch. No device collectives.

Device layout: activations are kept transposed ([dim, token], dim on
partitions) so every matmul contracts on the partition axis:
  Q^T = Wq_tile^T @ X^T          (stationary Wq tile, moving X^T tile)
  S^T[k,q] = K^T_tile^T @ Q^T    (contraction d=128, one matmul per tile)
  softmax over k (= partitions): exp on ACT, sums via ones-vector matmul
  ctx^T[d,q] = V_tile^T @ expS^T (V stationary [k_tok, d])
  out[tok,hid] = ctxT_tile^T @ Wo
Matmuls run in float32r (fp32 storage, reduced-precision multiply at 4x
fp32 speed); PSUM accumulation is fp32. RoPE pairs (i, i+64) live on
different partitions, so the half-swap is done with two SBUF->SBUF DMAs
and the rotation sign is baked into the host-built sin table.
"""
import numpy as np
from contextlib import ExitStack

import concourse.bass as bass
import concourse.tile as tile
from concourse import bacc, mybir
from concourse.bass_utils import run_bass_kernel_spmd

B, S, HID = 2, 2048, 2048
NH, NKV, D = 16, 4, 128
EPS = 1e-6
THETA = 1000000.0
SCALE = D ** -0.5

TP = 4                 # tensor-parallel groups (heads)
DP = 2                 # data-parallel over batch
HG = NH // TP          # q heads per core = 4
DQ = HG * D            # 512 q-proj cols per core
HALF = D // 2          # 64

F32 = mybir.dt.float32
F32R = mybir.dt.float32r

HT = HID // 128        # 16 hid tiles
TBS = 512              # token block size
NTB = S // TBS         # 4 token blocks
KT = S // 128          # 16 key tiles
QB = S // TBS          # 4 query blocks
NDT = HG + 2           # 6 projection outputs: q0..q3, k, v^T

STAGGER = 2            # AV matmul emission lag behind QK/exp

_cache = {}


def _build():
    nc = bacc.Bacc(None, target_bir_lowering=False, debug=False)

    xt = nc.dram_tensor("xt", [HID, S], F32R, kind="ExternalInput")
    wq = nc.dram_tensor("wq", [HID, DQ], F32R, kind="ExternalInput")
    wk = nc.dram_tensor("wk", [HID, D], F32R, kind="ExternalInput")
    wv = nc.dram_tensor("wv", [HID, D], F32R, kind="ExternalInput")
    wo = nc.dram_tensor("wo", [DQ, HID], F32R, kind="ExternalInput")
    cos2 = nc.dram_tensor("cos2", [D, S], F32, kind="ExternalInput")
    sin2 = nc.dram_tensor("sin2", [D, S], F32, kind="ExternalInput")
    qnw = nc.dram_tensor("qnw", [D, 1], F32, kind="ExternalInput")
    iden_d = nc.dram_tensor("iden", [128, 128], F32R, kind="ExternalInput")
    ones_d = nc.dram_tensor("ones", [128, 1], F32R, kind="ExternalInput")
    knw = nc.dram_tensor("knw", [D, 1], F32, kind="ExternalInput")
    out = nc.dram_tensor("out", [S, HID], F32, kind="ExternalOutput")

    with tile.TileContext(nc) as tc, ExitStack() as ctx:
        const = ctx.enter_context(tc.tile_pool(name="const", bufs=1))
        big = ctx.enter_context(tc.tile_pool(name="big", bufs=1))
        blk = ctx.enter_context(tc.tile_pool(name="blk", bufs=6))
        scratch = ctx.enter_context(tc.tile_pool(name="scratch", bufs=2))
        rows = ctx.enter_context(tc.tile_pool(name="rows", bufs=2))
        psum = ctx.enter_context(tc.tile_pool(name="psum", bufs=1, space="PSUM"))

        # ---- constants ----
        ident = const.tile([128, 128], F32R)
        nc.sync.dma_start(out=ident[:], in_=iden_d[:])
        ones_col = const.tile([128, 1], F32R)
        nc.sync.dma_start(out=ones_col[:], in_=ones_d[:])
        eps_row = const.tile([1, 1], F32)
        nc.vector.memset(eps_row, EPS)
        qnw_sb = const.tile([D, 1], F32)
        nc.sync.dma_start(out=qnw_sb[:], in_=qnw[:])
        knw_sb = const.tile([D, 1], F32)
        nc.sync.dma_start(out=knw_sb[:], in_=knw[:])

        # ---- resident weights / big activations (tag-shared slots) ----
        # "bigw": wq during phase A, wo during phase C (32KB/partition slot)
        wq_sb = big.tile([128, HT, DQ], F32R, tag="bigw")
        for ht in range(HT):
            nc.sync.dma_start(out=wq_sb[:, ht, :], in_=wq[ht * 128:(ht + 1) * 128, :])
        wk_sb = big.tile([128, HT, D], F32R, tag="wk")
        wv_sb = big.tile([128, HT, D], F32R, tag="wv")
        for ht in range(HT):
            nc.sync.dma_start(out=wk_sb[:, ht, :], in_=wk[ht * 128:(ht + 1) * 128, :])
            nc.sync.dma_start(out=wv_sb[:, ht, :], in_=wv[ht * 128:(ht + 1) * 128, :])
        cos_sb = big.tile([D, S], F32, tag="cos")
        nc.sync.dma_start(out=cos_sb[:], in_=cos2[:])
        sin_sb = big.tile([D, S], F32, tag="sin")
        nc.sync.dma_start(out=sin_sb[:], in_=sin2[:])

        qT = big.tile([D, HG, S], F32R, tag="qT")       # Q^T per head
        kT = big.tile([D, S], F32R, tag="kT")           # K^T
        vT = big.tile([D, S], F32R, tag="vT")           # V^T (pre-transpose)
        v_sb = big.tile([128, KT, D], F32R, tag="v")    # V [tok, d] tiles

        def stationary(ht, dt):
            if dt < HG:
                return wq_sb[:, ht, dt * D:(dt + 1) * D]
            if dt == HG:
                return wk_sb[:, ht, :]
            return wv_sb[:, ht, :]

        # ---- phase A: projections + rmsnorm + rope ----
        for tb in range(NTB):
            tsl = slice(tb * TBS, (tb + 1) * TBS)
            accs = [psum.tile([128, TBS], F32, tag=f"proj{dt}",
                              name=f"acc_{tb}_{dt}") for dt in range(NDT)]
            for ht in range(HT):
                xt_t = blk.tile([128, TBS], F32R, tag="blk", name=f"xt_{tb}_{ht}")
                nc.sync.dma_start(out=xt_t[:], in_=xt[ht * 128:(ht + 1) * 128, tsl])
                for dt in range(NDT):
                    nc.tensor.matmul(accs[dt][:], stationary(ht, dt), xt_t[:],
                                     start=(ht == 0), stop=(ht == HT - 1))
            for dt in range(NDT):
                acc = accs[dt]
                if dt == NDT - 1:
                    nc.scalar.copy(vT[:, tsl], acc[:])
                    continue
                w_ap = qnw_sb if dt < HG else knw_sb
                qn = scratch.tile([128, TBS], F32, tag="qn", name=f"qn_{tb}_{dt}")
                nc.scalar.activation(qn[:], acc[:],
                                     mybir.ActivationFunctionType.Copy,
                                     scale=w_ap[:])
                q2 = scratch.tile([128, TBS], F32R, tag="q2", name=f"q2_{tb}_{dt}")
                nc.scalar.square(q2[:], acc[:])
                ssq = psum.tile([1, TBS], F32, tag="small", bufs=2,
                                name=f"ssq_{tb}_{dt}")
                nc.tensor.matmul(ssq[:], ones_col[:], q2[:], start=True, stop=True)
                rstd = rows.tile([1, TBS], F32, tag="rstd", name=f"rstd_{tb}_{dt}")
                nc.scalar.activation(rstd[:], ssq[:],
                                     mybir.ActivationFunctionType.Sqrt,
                                     scale=1.0 / D, bias=eps_row[:])
                rstd2 = rows.tile([1, TBS], F32, tag="rstd2", name=f"rstd2_{tb}_{dt}")
                nc.vector.reciprocal(rstd2[:], rstd[:])
                rstdb = scratch.tile([128, TBS], F32, tag="bcast",
                                     name=f"rstdb_{tb}_{dt}")
                nc.gpsimd.partition_broadcast(rstdb[:], rstd2[:])
                # rope: swap halves via SBUF->SBUF DMA; sign baked into sin2
                xsw = scratch.tile([128, TBS], F32, tag="xsw", name=f"xsw_{tb}_{dt}")
                nc.sync.dma_start(out=xsw[0:HALF, :], in_=qn[HALF:D, :])
                nc.sync.dma_start(out=xsw[HALF:D, :], in_=qn[0:HALF, :])
                tmp = scratch.tile([128, TBS], F32, tag="tmp", name=f"tmp_{tb}_{dt}")
                nc.vector.tensor_mul(tmp[:], qn[:], cos_sb[:, tsl])
                sv = scratch.tile([128, TBS], F32, tag="sv", name=f"sv_{tb}_{dt}")
                nc.vector.tensor_mul(sv[:], xsw[:], sin_sb[:, tsl])
                qro = scratch.tile([128, TBS], F32, tag="qn", name=f"qro_{tb}_{dt}")
                nc.vector.tensor_add(qro[:], tmp[:], sv[:])
                dest = qT[:, dt, tsl] if dt < HG else kT[:, tsl]
                nc.vector.tensor_mul(dest, qro[:], rstdb[:])

        # ---- V^T -> V transpose (PE) ----
        for kt in range(KT):
            tp = psum.tile([128, 128], F32R, tag="proj0", name=f"tp_{kt}")
            nc.tensor.transpose(tp[:], vT[:, kt * 128:(kt + 1) * 128], ident[:])
            nc.vector.tensor_copy(v_sb[:, kt, :], tp[:])

        # ctx^T per head; slots reuse cos/sin/vT space (all dead by phase B)
        ctxT = [big.tile([D, S], F32R, tag=t, name=f"ctxT_{h}")
                for h, t in enumerate(["cos", "sin", "vT", "ctx3"])]

        # ---- phase B: attention per (head, query block) ----
        for h in range(HG):
            for qb in range(QB):
                qsl = slice(qb * TBS, (qb + 1) * TBS)
                ctx_ps = psum.tile([128, TBS], F32, tag="proj1",
                                   name=f"ctx_{h}_{qb}")
                sum_ps = psum.tile([1, TBS], F32, tag="small", bufs=2,
                                   name=f"sum_{h}_{qb}")
                pend = []

                def flush_one():
                    kt0, e0 = pend.pop(0)
                    nc.tensor.matmul(ctx_ps[:], v_sb[:, kt0, :], e0[:],
                                     start=(kt0 == 0), stop=(kt0 == KT - 1))
                    nc.tensor.matmul(sum_ps[:], ones_col[:], e0[:],
                                     start=(kt0 == 0), stop=(kt0 == KT - 1))

                for kt in range(KT):
                    s_ps = psum.tile([128, TBS], F32,
                                     tag=f"proj{2 + (kt % 3)}",
                                     name=f"s_{h}_{qb}_{kt}")
                    nc.tensor.matmul(s_ps[:], kT[:, kt * 128:(kt + 1) * 128],
                                     qT[:, h, qsl], start=True, stop=True)
                    e = blk.tile([128, TBS], F32R, tag="blk",
                                 name=f"e_{h}_{qb}_{kt}")
                    nc.scalar.activation(e[:], s_ps[:],
                                         mybir.ActivationFunctionType.Exp,
                                         scale=SCALE)
                    pend.append((kt, e))
                    if len(pend) > STAGGER:
                        flush_one()
                while pend:
                    flush_one()

                recip = rows.tile([1, TBS], F32, tag="recip",
                                  name=f"recip_{h}_{qb}")
                nc.vector.reciprocal(recip[:], sum_ps[:])
                recipb = scratch.tile([128, TBS], F32, tag="bcast",
                                      name=f"recipb_{h}_{qb}")
                nc.gpsimd.partition_broadcast(recipb[:], recip[:])
                nc.vector.tensor_mul(ctxT[h][:, qsl], ctx_ps[:], recipb[:])

        # ---- phase C: output projection ----
        wo_sb = big.tile([128, HG, HID], F32R, tag="bigw")
        for ct in range(HG):
            nc.sync.dma_start(out=wo_sb[:, ct, :],
                              in_=wo[ct * 128:(ct + 1) * 128, :])
        for tt in range(KT):
            for hc in range(HID // TBS):
                o_ps = psum.tile([128, TBS], F32, tag=f"proj{2 + (hc % 3)}",
                                 name=f"o_{tt}_{hc}")
                for ct in range(HG):
                    nc.tensor.matmul(
                        o_ps[:],
                        ctxT[ct][:, tt * 128:(tt + 1) * 128],
                        wo_sb[:, ct, hc * TBS:(hc + 1) * TBS],
                        start=(ct == 0), stop=(ct == HG - 1))
                o_sb = blk.tile([128, TBS], F32, tag="blk",
                                name=f"osb_{tt}_{hc}")
                nc.scalar.copy(o_sb[:], o_ps[:])
                nc.sync.dma_start(
                    out=out[tt * 128:(tt + 1) * 128, hc * TBS:(hc + 1) * TBS],
                    in_=o_sb[:])

    nc.compile()
    return nc


def _prep_inputs(hidden_states, positions, Wq, Wk, Wv, Wo, q_norm_w, k_norm_w):
    hidden_states = np.asarray(hidden_states, dtype=np.float32)
    positions = np.asarray(positions)
    Wq = np.asarray(Wq, dtype=np.float32)
    Wk = np.asarray(Wk, dtype=np.float32)
    Wv = np.asarray(Wv, dtype=np.float32)
    Wo = np.asarray(Wo, dtype=np.float32)
    q_norm_w = np.asarray(q_norm_w, dtype=np.float32)
    k_norm_w = np.asarray(k_norm_w, dtype=np.float32)

    inv_freq = THETA ** (-np.arange(HALF, dtype=np.float32) / HALF)
    in_maps = []
    for c in range(DP * TP):
        b, g = divmod(c, TP)
        freqs = positions[b].astype(np.float32)[:, None] * inv_freq[None, :]  # [S,64]
        cos = np.cos(freqs).T.astype(np.float32)      # [64, S]
        sin = np.sin(freqs).T.astype(np.float32)
        cos2 = np.ascontiguousarray(np.concatenate([cos, cos], axis=0))   # [128,S]
        sin2 = np.ascontiguousarray(np.concatenate([-sin, sin], axis=0))  # [128,S]
        in_maps.append({
            "xt": np.ascontiguousarray(hidden_states[b].T),
            "wq": np.ascontiguousarray(Wq[:, g * DQ:(g + 1) * DQ]),
            "wk": np.ascontiguousarray(Wk[:, g * D:(g + 1) * D]),
            "wv": np.ascontiguousarray(Wv[:, g * D:(g + 1) * D]),
            "wo": np.ascontiguousarray(Wo[g * DQ:(g + 1) * DQ, :]),
            "cos2": cos2,
            "sin2": sin2,
            "iden": np.eye(128, dtype=np.float32),
            "ones": np.ones((128, 1), dtype=np.float32),
            "qnw": np.ascontiguousarray(q_norm_w[:, None]),
            "knw": np.ascontiguousarray(k_norm_w[:, None]),
        })
    return in_maps


def _run(inputs, trace=False):
    if "nc" not in _cache:
        _cache["nc"] = _build()
    nc = _cache["nc"]
    in_maps = _prep_inputs(**inputs)
    res = run_bass_kernel_spmd(nc, in_maps, core_ids=list(range(DP * TP)),
                               trace=trace)
    out = np.zeros((B, S, HID), dtype=np.float32)
    for c in range(DP * TP):
        out[c // TP] += res.results[c]["out"]
    return out, res


def kernel(**inputs):
    out, _ = _run(inputs, trace=False)
    return out


# revision 8
# speedup vs baseline: 1.1090x; 1.1090x over previous
"""Trainium2 Bass kernel for nn_DFlashAttention_43774306681111.

Full-attention transformer block: QKV projection + per-head RMSNorm + neox
RoPE + GQA softmax attention (non-causal) + output projection.

Sharding (8 cores): 2-way data parallel over batch x 4-way tensor parallel
over heads. Core c handles batch c//4 and head group c%4 (q heads
4g..4g+3, kv head g). Each core computes a partial output [S, HID]
(its heads' contribution through Wo); the host sums the 4 partials per
batch. No device collectives.

Device layout: activations are kept transposed ([dim, token], dim on
partitions) so every matmul contracts on the partition axis:
  Q^T = Wq_tile^T @ X^T          (stationary Wq tile, moving X^T tile)
  S^T[k,q] = K^T_tile^T @ Q^T    (contraction d=128, one matmul per tile)
  softmax over k (= partitions): exp on ACT, sums via ones-vector matmul
  ctx^T[d,q] = V_tile^T @ expS^T (V stationary [k_tok, d])
  out[tok,hid] = ctxT_tile^T @ Wo
Matmuls run in float32r (fp32 storage, reduced-precision multiply);
PSUM accumulation is fp32. RoPE pairs (i, i+64) live on different
partitions, so the half-swap is done with two SBUF->SBUF DMAs and the
rotation sign is baked into the host-built sin table.

The attention loop is qb-major and the Wo matmuls for a finished query
block are deferred into the middle of the next block's QK stream, so the
PE never waits on a phase barrier.
"""
import numpy as np
from contextlib import ExitStack

import concourse.bass as bass
import concourse.tile as tile
from concourse import bacc, mybir
from concourse.bass_utils import run_bass_kernel_spmd

B, S, HID = 2, 2048, 2048
NH, NKV, D = 16, 4, 128
EPS = 1e-6
THETA = 1000000.0
SCALE = D ** -0.5

TP = 4                 # tensor-parallel groups (heads)
DP = 2                 # data-parallel over batch
HG = NH // TP          # q heads per core = 4
DQ = HG * D            # 512 q-proj cols per core
HALF = D // 2          # 64

F32 = mybir.dt.float32
F32R = mybir.dt.float32r
BF16 = mybir.dt.bfloat16

MM_DT = F32R           # matmul operand dtype (F32R or BF16)
MM_NP = np.float32     # host dtype fed into MM_DT dram tensors

HT = HID // 128        # 16 hid tiles
TBS = 512              # token block size
NTB = S // TBS         # 4 token blocks
KT = S // 128          # 16 key tiles
QB = S // TBS          # 4 query blocks
NDT = HG + 2           # 6 projection outputs: q0..q3, k, v^T

STAGGER = 3            # AV matmul emission lag behind QK/exp
WO_FLUSH_KT = 4        # emit deferred Wo matmuls after this many QK tiles

_cache = {}


def _build():
    nc = bacc.Bacc(None, target_bir_lowering=False, debug=False)

    xt = nc.dram_tensor("xt", [HID, S], MM_DT, kind="ExternalInput")
    wq = nc.dram_tensor("wq", [HID, DQ], MM_DT, kind="ExternalInput")
    wk = nc.dram_tensor("wk", [HID, D], MM_DT, kind="ExternalInput")
    wv = nc.dram_tensor("wv", [HID, D], MM_DT, kind="ExternalInput")
    wo = nc.dram_tensor("wo", [DQ, HID], MM_DT, kind="ExternalInput")
    cos2 = nc.dram_tensor("cos2", [D, S], F32, kind="ExternalInput")
    sin2 = nc.dram_tensor("sin2", [D, S], F32, kind="ExternalInput")
    qnw = nc.dram_tensor("qnw", [D, 1], F32, kind="ExternalInput")
    knw = nc.dram_tensor("knw", [D, 1], F32, kind="ExternalInput")
    iden_d = nc.dram_tensor("iden", [128, 128], MM_DT, kind="ExternalInput")
    ones_d = nc.dram_tensor("ones", [128, 1], MM_DT, kind="ExternalInput")
    out = nc.dram_tensor("out", [S, HID], F32, kind="ExternalOutput")

    with tile.TileContext(nc) as tc, ExitStack() as ctx:
        const = ctx.enter_context(tc.tile_pool(name="const", bufs=1))
        big = ctx.enter_context(tc.tile_pool(name="big", bufs=1))
        blk = ctx.enter_context(tc.tile_pool(name="blk", bufs=8))
        outp = ctx.enter_context(tc.tile_pool(name="outp", bufs=3))
        scratch = ctx.enter_context(tc.tile_pool(name="scratch", bufs=2))
        rows = ctx.enter_context(tc.tile_pool(name="rows", bufs=2))
        psum = ctx.enter_context(tc.tile_pool(name="psum", bufs=1, space="PSUM"))

        # ---- constants ----
        ident = const.tile([128, 128], MM_DT)
        nc.sync.dma_start(out=ident[:], in_=iden_d[:])
        ones_col = const.tile([128, 1], MM_DT)
        nc.sync.dma_start(out=ones_col[:], in_=ones_d[:])
        eps_row = const.tile([1, 1], F32)
        nc.vector.memset(eps_row, EPS)
        qnw_sb = const.tile([D, 1], F32)
        nc.sync.dma_start(out=qnw_sb[:], in_=qnw[:])
        knw_sb = const.tile([D, 1], F32)
        nc.sync.dma_start(out=knw_sb[:], in_=knw[:])

        # ---- resident weights / big activations (tag-shared slots) ----
        # weight DMAs are emitted inside the tb==0 loop, interleaved per hid
        # tile with the xt loads, so the first matmul starts ~2us in instead
        # of waiting for all weights.
        wq_sb = big.tile([128, HT, DQ], MM_DT, tag="bigw")
        wk_sb = big.tile([128, HT, D], MM_DT, tag="wk")
        wv_sb = big.tile([128, HT, D], MM_DT, tag="wv")
        cos_sb = big.tile([D, S], F32, tag="cos")
        sin_sb = big.tile([D, S], F32, tag="sin")

        qT = big.tile([D, HG, S], MM_DT, tag="qT")       # Q^T per head
        kT = big.tile([D, S], MM_DT, tag="kT")           # K^T
        vT = big.tile([D, S], MM_DT, tag="vT")           # V^T (pre-transpose)
        v_sb = big.tile([128, KT, D], MM_DT, tag="v")    # V [tok, d] tiles

        def stationary(ht, dt):
            if dt < HG:
                return wq_sb[:, ht, dt * D:(dt + 1) * D]
            if dt == HG:
                return wk_sb[:, ht, :]
            return wv_sb[:, ht, :]

        # ---- phase A: projections + rmsnorm + rope ----
        for tb in range(NTB):
            tsl = slice(tb * TBS, (tb + 1) * TBS)
            # rope tables arrive chunk-by-chunk, just ahead of each eviction
            nc.sync.dma_start(out=cos_sb[:, tsl], in_=cos2[:, tsl])
            nc.sync.dma_start(out=sin_sb[:, tsl], in_=sin2[:, tsl])
            accs = [psum.tile([128, TBS], F32, tag=f"proj{dt}",
                              name=f"acc_{tb}_{dt}") for dt in range(NDT)]
            for ht in range(HT):
                if tb == 0:
                    hsl = slice(ht * 128, (ht + 1) * 128)
                    nc.sync.dma_start(out=wq_sb[:, ht, :], in_=wq[hsl, :])
                    nc.sync.dma_start(out=wk_sb[:, ht, :], in_=wk[hsl, :])
                    nc.sync.dma_start(out=wv_sb[:, ht, :], in_=wv[hsl, :])
                xt_t = blk.tile([128, TBS], MM_DT, tag="blk", name=f"xt_{tb}_{ht}")
                nc.sync.dma_start(out=xt_t[:], in_=xt[ht * 128:(ht + 1) * 128, tsl])
                for dt in range(NDT):
                    nc.tensor.matmul(accs[dt][:], stationary(ht, dt), xt_t[:],
                                     start=(ht == 0), stop=(ht == HT - 1))
            for dt in range(NDT):
                acc = accs[dt]
                if dt == NDT - 1:
                    nc.scalar.copy(vT[:, tsl], acc[:])
                    continue
                w_ap = qnw_sb if dt < HG else knw_sb
                qn = scratch.tile([128, TBS], F32, tag="qn", name=f"qn_{tb}_{dt}")
                nc.scalar.activation(qn[:], acc[:],
                                     mybir.ActivationFunctionType.Copy,
                                     scale=w_ap[:])
                q2 = scratch.tile([128, TBS], MM_DT, tag="q2", name=f"q2_{tb}_{dt}")
                nc.scalar.square(q2[:], acc[:])
                ssq = psum.tile([1, TBS], F32, tag="small", bufs=1,
                                name=f"ssq_{tb}_{dt}")
                nc.tensor.matmul(ssq[:], ones_col[:], q2[:],
                                 start=True, stop=True)
                rstd = rows.tile([1, TBS], F32, tag="rstd", name=f"rstd_{tb}_{dt}")
                nc.scalar.activation(rstd[:], ssq[:],
                                     mybir.ActivationFunctionType.Sqrt,
                                     scale=1.0 / D, bias=eps_row[:])
                rstd2 = rows.tile([1, TBS], F32, tag="rstd2", name=f"rstd2_{tb}_{dt}")
                nc.vector.reciprocal(rstd2[:], rstd[:])
                rstdb = scratch.tile([128, TBS], F32, tag="bcast",
                                     name=f"rstdb_{tb}_{dt}")
                nc.gpsimd.partition_broadcast(rstdb[:], rstd2[:])
                # rope: swap halves via SBUF->SBUF DMA; sign baked into sin2
                xsw = scratch.tile([128, TBS], F32, tag="xsw", name=f"xsw_{tb}_{dt}")
                nc.sync.dma_start(out=xsw[0:HALF, :], in_=qn[HALF:D, :])
                nc.sync.dma_start(out=xsw[HALF:D, :], in_=qn[0:HALF, :])
                tmp = scratch.tile([128, TBS], F32, tag="tmp", name=f"tmp_{tb}_{dt}")
                nc.vector.tensor_mul(tmp[:], qn[:], cos_sb[:, tsl])
                sv = scratch.tile([128, TBS], F32, tag="sv", name=f"sv_{tb}_{dt}")
                nc.vector.tensor_mul(sv[:], xsw[:], sin_sb[:, tsl])
                qro = scratch.tile([128, TBS], F32, tag="qn", name=f"qro_{tb}_{dt}")
                nc.vector.tensor_add(qro[:], tmp[:], sv[:])
                dest = qT[:, dt, tsl] if dt < HG else kT[:, tsl]
                nc.vector.tensor_mul(dest, qro[:], rstdb[:])

        # ---- V^T -> V transpose (PE) ----
        for kt in range(KT):
            tp = psum.tile([128, 128], MM_DT, tag="proj0", name=f"tp_{kt}")
            nc.tensor.transpose(tp[:], vT[:, kt * 128:(kt + 1) * 128], ident[:])
            nc.vector.tensor_copy(v_sb[:, kt, :], tp[:])

        # ctx^T per head; slots reuse cos/sin/vT space (all dead by phase B)
        ctxT = [big.tile([D, S], MM_DT, tag=t, name=f"ctxT_{h}")
                for h, t in enumerate(["cos", "sin", "vT", "ctx3"])]

        # wo loads overlap the first attention blocks ("bigw" frees after
        # the last projection matmul)
        wo_sb = big.tile([128, HG, HID], MM_DT, tag="bigw")
        for ct in range(HG):
            nc.sync.dma_start(out=wo_sb[:, ct, :],
                              in_=wo[ct * 128:(ct + 1) * 128, :])

        # ---- phase B: attention, qb-major, with Wo folded in ----
        pending_wo = []

        def emit_wo(qb):
            qb_out = []
            for tt in range(qb * (TBS // 128), (qb + 1) * (TBS // 128)):
                for hc in range(HID // TBS):
                    def thunk(tt=tt, hc=hc):
                        o_ps = psum.tile([128, TBS], F32,
                                         tag=f"proj{[0, 4][(tt * 4 + hc) % 2]}",
                                         name=f"o_{tt}_{hc}")
                        for ct in range(HG):
                            nc.tensor.matmul(
                                o_ps[:],
                                ctxT[ct][:, tt * 128:(tt + 1) * 128],
                                wo_sb[:, ct, hc * TBS:(hc + 1) * TBS],
                                start=(ct == 0), stop=(ct == HG - 1))
                        o_sb = outp.tile([128, TBS], F32, tag="osb",
                                         name=f"osb_{tt}_{hc}")
                        nc.scalar.copy(o_sb[:], o_ps[:])
                        nc.sync.dma_start(
                            out=out[tt * 128:(tt + 1) * 128,
                                    hc * TBS:(hc + 1) * TBS],
                            in_=o_sb[:])
                    qb_out.append(thunk)
            return qb_out

        def flush_wo():
            while pending_wo:
                pending_wo.pop(0)()

        for qb in range(QB):
            qsl = slice(qb * TBS, (qb + 1) * TBS)
            for h in range(HG):
                blk_i = qb * HG + h
                ctx_ps = psum.tile([128, TBS], F32,
                                   tag=f"proj{[1, 5][blk_i % 2]}",
                                   name=f"ctx_{h}_{qb}")
                sum_ps = psum.tile([1, TBS], F32, tag="small", bufs=1,
                                   name=f"sum_{h}_{qb}")
                pend = []

                def flush_one():
                    kt0, e0 = pend.pop(0)
                    nc.tensor.matmul(ctx_ps[:], v_sb[:, kt0, :], e0[:],
                                     start=(kt0 == 0), stop=(kt0 == KT - 1))
                    nc.tensor.matmul(sum_ps[:], ones_col[:], e0[:],
                                     start=(kt0 == 0), stop=(kt0 == KT - 1))

                for kt in range(KT):
                    s_ps = psum.tile([128, TBS], F32,
                                     tag=f"proj{2 + (kt % 2)}",
                                     name=f"s_{h}_{qb}_{kt}")
                    nc.tensor.matmul(s_ps[:], kT[:, kt * 128:(kt + 1) * 128],
                                     qT[:, h, qsl], start=True, stop=True)
                    e = blk.tile([128, TBS], MM_DT, tag="blk",
                                 name=f"e_{h}_{qb}_{kt}")
                    nc.scalar.activation(e[:], s_ps[:],
                                         mybir.ActivationFunctionType.Exp,
                                         scale=SCALE)
                    pend.append((kt, e))
                    if len(pend) > STAGGER:
                        flush_one()
                    if h == 0 and kt == WO_FLUSH_KT:
                        flush_wo()  # previous qb's Wo rides in the QK stream
                while pend:
                    flush_one()

                recip = rows.tile([1, TBS], F32, tag="recip",
                                  name=f"recip_{h}_{qb}")
                nc.vector.reciprocal(recip[:], sum_ps[:])
                recipb = scratch.tile([128, TBS], F32, tag="bcast",
                                      name=f"recipb_{h}_{qb}")
                nc.gpsimd.partition_broadcast(recipb[:], recip[:])
                nc.vector.tensor_mul(ctxT[h][:, qsl], ctx_ps[:], recipb[:])
            pending_wo = emit_wo(qb)
        flush_wo()

    nc.compile()
    return nc


def _prep_inputs(hidden_states, positions, Wq, Wk, Wv, Wo, q_norm_w, k_norm_w):
    hidden_states = np.asarray(hidden_states, dtype=np.float32)
    positions = np.asarray(positions)
    Wq = np.asarray(Wq, dtype=np.float32)
    Wk = np.asarray(Wk, dtype=np.float32)
    Wv = np.asarray(Wv, dtype=np.float32)
    Wo = np.asarray(Wo, dtype=np.float32)
    q_norm_w = np.asarray(q_norm_w, dtype=np.float32)
    k_norm_w = np.asarray(k_norm_w, dtype=np.float32)

    inv_freq = THETA ** (-np.arange(HALF, dtype=np.float32) / HALF)
    in_maps = []
    for c in range(DP * TP):
        b, g = divmod(c, TP)
        freqs = positions[b].astype(np.float32)[:, None] * inv_freq[None, :]  # [S,64]
        cos = np.cos(freqs).T.astype(np.float32)      # [64, S]
        sin = np.sin(freqs).T.astype(np.float32)
        cos2 = np.ascontiguousarray(np.concatenate([cos, cos], axis=0))   # [128,S]
        sin2 = np.ascontiguousarray(np.concatenate([-sin, sin], axis=0))  # [128,S]
        in_maps.append({
            "xt": np.ascontiguousarray(hidden_states[b].T).astype(MM_NP),
            "wq": np.ascontiguousarray(Wq[:, g * DQ:(g + 1) * DQ]).astype(MM_NP),
            "wk": np.ascontiguousarray(Wk[:, g * D:(g + 1) * D]).astype(MM_NP),
            "wv": np.ascontiguousarray(Wv[:, g * D:(g + 1) * D]).astype(MM_NP),
            "wo": np.ascontiguousarray(Wo[g * DQ:(g + 1) * DQ, :]).astype(MM_NP),
            "cos2": cos2,
            "sin2": sin2,
            "qnw": np.ascontiguousarray(q_norm_w[:, None]),
            "knw": np.ascontiguousarray(k_norm_w[:, None]),
            "iden": np.eye(128, dtype=MM_NP),
            "ones": np.ones((128, 1), dtype=MM_NP),
        })
    return in_maps


def _run(inputs, trace=False):
    if "nc" not in _cache:
        _cache["nc"] = _build()
    nc = _cache["nc"]
    in_maps = _prep_inputs(**inputs)
    res = run_bass_kernel_spmd(nc, in_maps, core_ids=list(range(DP * TP)),
                               trace=trace)
    out = np.zeros((B, S, HID), dtype=np.float32)
    for c in range(DP * TP):
        out[c // TP] += res.results[c]["out"]
    return out, res


def kernel(**inputs):
    out, _ = _run(inputs, trace=False)
    return out


# revision 12
# speedup vs baseline: 1.1331x; 1.0217x over previous
"""Trainium2 Bass kernel for nn_DFlashAttention_43774306681111.

Full-attention transformer block: QKV projection + per-head RMSNorm + neox
RoPE + GQA softmax attention (non-causal) + output projection.

Sharding (8 cores): 2-way data parallel over batch x 4-way tensor parallel
over heads. Core c handles batch c//4 and head group c%4 (q heads
4g..4g+3, kv head g). Each core computes a partial output [S, HID]
(its heads' contribution through Wo); the host sums the 4 partials per
batch. No device collectives.

Device layout: activations are kept transposed ([dim, token], dim on
partitions) so every matmul contracts on the partition axis:
  Q^T = Wq_tile^T @ X^T          (stationary Wq tile, moving X^T tile)
  S^T[k,q] = K^T_tile^T @ Q^T    (contraction d=128, one matmul per tile)
  softmax over k (= partitions): exp on ACT, sums via ones-vector matmul
  ctx^T[d,q] = V_tile^T @ expS^T (V stationary [k_tok, d])
  out[tok,hid] = ctxT_tile^T @ Wo
Matmuls run in float32r (fp32 storage, reduced-precision multiply);
PSUM accumulation is fp32. RoPE pairs (i, i+64) live on different
partitions, so the half-swap is done with two SBUF->SBUF DMAs and the
rotation sign is baked into the host-built sin table.

The attention loop is qb-major and the Wo matmuls for a finished query
block are deferred into the middle of the next block's QK stream, so the
PE never waits on a phase barrier.
"""
import numpy as np
from contextlib import ExitStack

import concourse.bass as bass
import concourse.tile as tile
from concourse import bacc, mybir
from concourse.bass_utils import run_bass_kernel_spmd

B, S, HID = 2, 2048, 2048
NH, NKV, D = 16, 4, 128
EPS = 1e-6
THETA = 1000000.0
SCALE = D ** -0.5

TP = 4                 # tensor-parallel groups (heads)
DP = 2                 # data-parallel over batch
HG = NH // TP          # q heads per core = 4
DQ = HG * D            # 512 q-proj cols per core
HALF = D // 2          # 64

F32 = mybir.dt.float32
F32R = mybir.dt.float32r
BF16 = mybir.dt.bfloat16

MM_DT = F32R           # matmul operand dtype (F32R or BF16)
MM_NP = np.float32     # host dtype fed into MM_DT dram tensors

HT = HID // 128        # 16 hid tiles
TBS = 512              # token block size
NTB = S // TBS         # 4 token blocks
KT = S // 128          # 16 key tiles
QB = S // TBS          # 4 query blocks
NDT = HG + 2           # 6 projection outputs: q0..q3, k, v^T

STAGGER = 3            # AV matmul emission lag behind QK/exp
WO_FLUSH_KT = 4        # emit deferred Wo matmuls after this many QK tiles

_cache = {}


def _build():
    nc = bacc.Bacc(None, target_bir_lowering=False, debug=False)

    xt = nc.dram_tensor("xt", [HID, S], MM_DT, kind="ExternalInput")
    wq = nc.dram_tensor("wq", [HID, DQ], MM_DT, kind="ExternalInput")
    wk = nc.dram_tensor("wk", [HID, D], MM_DT, kind="ExternalInput")
    wv = nc.dram_tensor("wv", [HID, D], MM_DT, kind="ExternalInput")
    wo = nc.dram_tensor("wo", [DQ, HID], MM_DT, kind="ExternalInput")
    cos2 = nc.dram_tensor("cos2", [D, S], F32, kind="ExternalInput")
    sin2 = nc.dram_tensor("sin2", [D, S], F32, kind="ExternalInput")
    qnw = nc.dram_tensor("qnw", [D, 1], F32, kind="ExternalInput")
    knw = nc.dram_tensor("knw", [D, 1], F32, kind="ExternalInput")
    iden_d = nc.dram_tensor("iden", [128, 128], MM_DT, kind="ExternalInput")
    ones_d = nc.dram_tensor("ones", [128, 1], MM_DT, kind="ExternalInput")
    out = nc.dram_tensor("out", [S, HID], F32, kind="ExternalOutput")

    with tile.TileContext(nc) as tc, ExitStack() as ctx:
        const = ctx.enter_context(tc.tile_pool(name="const", bufs=1))
        big = ctx.enter_context(tc.tile_pool(name="big", bufs=1))
        blk = ctx.enter_context(tc.tile_pool(name="blk", bufs=5))
        outp = ctx.enter_context(tc.tile_pool(name="outp", bufs=2))
        scratch = ctx.enter_context(tc.tile_pool(name="scratch", bufs=2))
        rows = ctx.enter_context(tc.tile_pool(name="rows", bufs=2))
        psum = ctx.enter_context(tc.tile_pool(name="psum", bufs=1, space="PSUM"))

        # ---- constants ----
        ident = const.tile([128, 128], MM_DT)
        nc.sync.dma_start(out=ident[:], in_=iden_d[:])
        ones_col = const.tile([128, 1], MM_DT)
        nc.sync.dma_start(out=ones_col[:], in_=ones_d[:])
        eps_row = const.tile([1, 1], F32)
        nc.vector.memset(eps_row, EPS)
        qnw_sb = const.tile([D, 1], F32)
        nc.sync.dma_start(out=qnw_sb[:], in_=qnw[:])
        knw_sb = const.tile([D, 1], F32)
        nc.sync.dma_start(out=knw_sb[:], in_=knw[:])

        # ---- resident weights / big activations (tag-shared slots) ----
        # weight DMAs are emitted inside the tb==0 loop, interleaved per hid
        # tile with the xt loads, so the first matmul starts ~2us in instead
        # of waiting for all weights.
        wq_sb = big.tile([128, HT, DQ], MM_DT, tag="bigw")
        wk_sb = big.tile([128, HT, D], MM_DT, tag="wk")
        wv_sb = big.tile([128, HT, D], MM_DT, tag="wv")
        cos_sb = big.tile([D, S], F32, tag="cos")
        sin_sb = big.tile([D, S], F32, tag="sin")

        qT = big.tile([D, HG, S], MM_DT, tag="qT")       # Q^T per head
        kT = big.tile([D, S], MM_DT, tag="kT")           # K^T
        vT = big.tile([D, S], MM_DT, tag="vT")           # V^T (pre-transpose)
        v_sb = big.tile([128, KT, D], MM_DT, tag="v")    # V [tok, d] tiles

        def stationary(ht, dt):
            if dt < HG:
                return wq_sb[:, ht, dt * D:(dt + 1) * D]
            if dt == HG:
                return wk_sb[:, ht, :]
            return wv_sb[:, ht, :]

        # ---- phase A: projections + rmsnorm + rope ----
        # eviction tails (rmsnorm + rope) are deferred into the next token
        # block's matmul stream so their ssq matmuls never stall the PE
        pending_evict = []

        def flush_evict():
            while pending_evict:
                pending_evict.pop(0)()

        for tb in range(NTB):
            tsl = slice(tb * TBS, (tb + 1) * TBS)
            accs = [psum.tile([128, TBS], F32, tag=f"p{'ABCDEF'[dt]}",
                              name=f"acc_{tb}_{dt}") for dt in range(NDT)]
            for ht in range(HT):
                if tb == 0:
                    hsl = slice(ht * 128, (ht + 1) * 128)
                    nc.sync.dma_start(out=wq_sb[:, ht, :], in_=wq[hsl, :])
                    nc.sync.dma_start(out=wk_sb[:, ht, :], in_=wk[hsl, :])
                    nc.sync.dma_start(out=wv_sb[:, ht, :], in_=wv[hsl, :])
                if ht == 1:
                    # rope tables arrive ahead of this tb's (deferred) eviction
                    nc.sync.dma_start(out=cos_sb[:, tsl], in_=cos2[:, tsl])
                    nc.sync.dma_start(out=sin_sb[:, tsl], in_=sin2[:, tsl])
                if ht == 2:
                    flush_evict()  # previous tb's eviction tail rides this stream
                xt_t = blk.tile([128, TBS], MM_DT, tag="blk", name=f"xt_{tb}_{ht}")
                nc.sync.dma_start(out=xt_t[:], in_=xt[ht * 128:(ht + 1) * 128, tsl])
                for dt in range(NDT):
                    nc.tensor.matmul(accs[dt][:], stationary(ht, dt), xt_t[:],
                                     start=(ht == 0), stop=(ht == HT - 1))
            for dt in range(NDT):
                acc = accs[dt]
                if dt == NDT - 1:
                    nc.scalar.copy(vT[:, tsl], acc[:])
                    continue
                w_ap = qnw_sb if dt < HG else knw_sb
                # single psum read (alternating engines) frees the bank fast;
                # all other eviction work reads the SBUF copy and is deferred
                raw = scratch.tile([128, TBS], F32, tag="raw", bufs=7,
                                   name=f"raw_{tb}_{dt}")
                if dt % 2 == 0:
                    nc.vector.tensor_copy(raw[:], acc[:])
                else:
                    nc.scalar.copy(raw[:], acc[:])

                def evict_tail(tb=tb, dt=dt, raw=raw, w_ap=w_ap, tsl=tsl):
                    qn = scratch.tile([128, TBS], F32, tag="qn", bufs=2,
                                      name=f"qn_{tb}_{dt}")
                    nc.scalar.activation(qn[:], raw[:],
                                         mybir.ActivationFunctionType.Copy,
                                         scale=w_ap[:])
                    q2 = scratch.tile([128, TBS], MM_DT, tag="q2", bufs=2,
                                      name=f"q2_{tb}_{dt}")
                    nc.vector.tensor_mul(q2[:], raw[:], raw[:])
                    ssq = psum.tile([1, TBS], F32, tag="small", bufs=1,
                                    name=f"ssq_{tb}_{dt}")
                    nc.tensor.matmul(ssq[:], ones_col[:], q2[:],
                                     start=True, stop=True)
                    rstd = rows.tile([1, TBS], F32, tag="rstd", bufs=2,
                                     name=f"rstd_{tb}_{dt}")
                    nc.scalar.activation(rstd[:], ssq[:],
                                         mybir.ActivationFunctionType.Sqrt,
                                         scale=1.0 / D, bias=eps_row[:])
                    nc.vector.reciprocal(rstd[:], rstd[:])
                    rstdb = scratch.tile([128, TBS], F32, tag="bcast", bufs=3,
                                         name=f"rstdb_{tb}_{dt}")
                    nc.gpsimd.partition_broadcast(rstdb[:], rstd[:])
                    # rope: swap halves via SBUF->SBUF DMA; sign baked in sin2
                    xsw = scratch.tile([128, TBS], F32, tag="xsw", bufs=3,
                                       name=f"xsw_{tb}_{dt}")
                    nc.sync.dma_start(out=xsw[0:HALF, :], in_=qn[HALF:D, :])
                    nc.sync.dma_start(out=xsw[HALF:D, :], in_=qn[0:HALF, :])
                    tmp = scratch.tile([128, TBS], F32, tag="tmp", bufs=2,
                                       name=f"tmp_{tb}_{dt}")
                    nc.vector.tensor_mul(tmp[:], qn[:], cos_sb[:, tsl])
                    sv = scratch.tile([128, TBS], F32, tag="sv", bufs=2,
                                      name=f"sv_{tb}_{dt}")
                    nc.vector.tensor_mul(sv[:], xsw[:], sin_sb[:, tsl])
                    qro = scratch.tile([128, TBS], F32, tag="qro", bufs=2,
                                       name=f"qro_{tb}_{dt}")
                    nc.vector.tensor_add(qro[:], tmp[:], sv[:])
                    dest = qT[:, dt, tsl] if dt < HG else kT[:, tsl]
                    nc.vector.tensor_mul(dest, qro[:], rstdb[:])
                pending_evict.append(evict_tail)

        # ---- V^T -> V transpose (PE) ----
        for kt in range(KT):
            tp = psum.tile([128, 128], MM_DT, tag="pE", name=f"tp_{kt}")
            nc.tensor.transpose(tp[:], vT[:, kt * 128:(kt + 1) * 128], ident[:])
            nc.vector.tensor_copy(v_sb[:, kt, :], tp[:])
            if kt == 2:
                flush_evict()  # last tb's eviction tail rides the transposes

        # ctx^T per head; slots reuse cos/sin/vT space (all dead by phase B)
        ctxT = [big.tile([D, S], MM_DT, tag=t, name=f"ctxT_{h}")
                for h, t in enumerate(["cos", "sin", "vT", "ctx3"])]

        # wo loads overlap the first attention blocks ("bigw" frees after
        # the last projection matmul)
        wo_sb = big.tile([128, HG, HID], MM_DT, tag="bigw")
        for ct in range(HG):
            nc.sync.dma_start(out=wo_sb[:, ct, :],
                              in_=wo[ct * 128:(ct + 1) * 128, :])

        # ---- phase B: attention, qb-major, with Wo folded in ----
        pending_wo = []

        def emit_wo(qb):
            qb_out = []
            for tt in range(qb * (TBS // 128), (qb + 1) * (TBS // 128)):
                for hc in range(HID // TBS):
                    def thunk(tt=tt, hc=hc):
                        o_ps = psum.tile([128, TBS], F32,
                                         tag=f"p{'EF'[(tt * 4 + hc) % 2]}",
                                         name=f"o_{tt}_{hc}")
                        for ct in range(HG):
                            nc.tensor.matmul(
                                o_ps[:],
                                ctxT[ct][:, tt * 128:(tt + 1) * 128],
                                wo_sb[:, ct, hc * TBS:(hc + 1) * TBS],
                                start=(ct == 0), stop=(ct == HG - 1))
                        o_sb = outp.tile([128, TBS], F32, tag="osb",
                                         name=f"osb_{tt}_{hc}")
                        nc.scalar.copy(o_sb[:], o_ps[:])
                        nc.sync.dma_start(
                            out=out[tt * 128:(tt + 1) * 128,
                                    hc * TBS:(hc + 1) * TBS],
                            in_=o_sb[:])
                    qb_out.append(thunk)
            return qb_out

        def flush_wo():
            while pending_wo:
                pending_wo.pop(0)()

        for qb in range(QB):
            qsl = slice(qb * TBS, (qb + 1) * TBS)
            for h in range(HG):
                blk_i = qb * HG + h
                ctx_ps = psum.tile([128, TBS], F32,
                                   tag=f"p{'CD'[blk_i % 2]}",
                                   name=f"ctx_{h}_{qb}")
                sum_ps = psum.tile([1, TBS], F32, tag="small", bufs=1,
                                   name=f"sum_{h}_{qb}")
                pend = []

                def flush_one():
                    kt0, e0 = pend.pop(0)
                    nc.tensor.matmul(ctx_ps[:], v_sb[:, kt0, :], e0[:],
                                     start=(kt0 == 0), stop=(kt0 == KT - 1))
                    nc.tensor.matmul(sum_ps[:], ones_col[:], e0[:],
                                     start=(kt0 == 0), stop=(kt0 == KT - 1))

                for kt in range(KT):
                    s_ps = psum.tile([128, TBS], F32,
                                     tag=f"p{'ABG'[kt % 3]}",
                                     name=f"s_{h}_{qb}_{kt}")
                    nc.tensor.matmul(s_ps[:], kT[:, kt * 128:(kt + 1) * 128],
                                     qT[:, h, qsl], start=True, stop=True)
                    e = blk.tile([128, TBS], MM_DT, tag="blk",
                                 name=f"e_{h}_{qb}_{kt}")
                    nc.scalar.activation(e[:], s_ps[:],
                                         mybir.ActivationFunctionType.Exp,
                                         scale=SCALE)
                    pend.append((kt, e))
                    if len(pend) > STAGGER:
                        flush_one()
                    if h == 0 and kt == WO_FLUSH_KT:
                        flush_wo()  # previous qb's Wo rides in the QK stream
                while pend:
                    flush_one()

                recip = rows.tile([1, TBS], F32, tag="recip",
                                  name=f"recip_{h}_{qb}")
                nc.vector.reciprocal(recip[:], sum_ps[:])
                recipb = scratch.tile([128, TBS], F32, tag="bcast", bufs=3,
                                      name=f"recipb_{h}_{qb}")
                nc.gpsimd.partition_broadcast(recipb[:], recip[:])
                nc.vector.tensor_mul(ctxT[h][:, qsl], ctx_ps[:], recipb[:])
            pending_wo = emit_wo(qb)
        flush_wo()

    nc.compile()
    return nc


def _prep_inputs(hidden_states, positions, Wq, Wk, Wv, Wo, q_norm_w, k_norm_w):
    hidden_states = np.asarray(hidden_states, dtype=np.float32)
    positions = np.asarray(positions)
    Wq = np.asarray(Wq, dtype=np.float32)
    Wk = np.asarray(Wk, dtype=np.float32)
    Wv = np.asarray(Wv, dtype=np.float32)
    Wo = np.asarray(Wo, dtype=np.float32)
    q_norm_w = np.asarray(q_norm_w, dtype=np.float32)
    k_norm_w = np.asarray(k_norm_w, dtype=np.float32)

    inv_freq = THETA ** (-np.arange(HALF, dtype=np.float32) / HALF)
    in_maps = []
    for c in range(DP * TP):
        b, g = divmod(c, TP)
        freqs = positions[b].astype(np.float32)[:, None] * inv_freq[None, :]  # [S,64]
        cos = np.cos(freqs).T.astype(np.float32)      # [64, S]
        sin = np.sin(freqs).T.astype(np.float32)
        cos2 = np.ascontiguousarray(np.concatenate([cos, cos], axis=0))   # [128,S]
        sin2 = np.ascontiguousarray(np.concatenate([-sin, sin], axis=0))  # [128,S]
        in_maps.append({
            "xt": np.ascontiguousarray(hidden_states[b].T).astype(MM_NP),
            "wq": np.ascontiguousarray(Wq[:, g * DQ:(g + 1) * DQ]).astype(MM_NP),
            "wk": np.ascontiguousarray(Wk[:, g * D:(g + 1) * D]).astype(MM_NP),
            "wv": np.ascontiguousarray(Wv[:, g * D:(g + 1) * D]).astype(MM_NP),
            "wo": np.ascontiguousarray(Wo[g * DQ:(g + 1) * DQ, :]).astype(MM_NP),
            "cos2": cos2,
            "sin2": sin2,
            "qnw": np.ascontiguousarray(q_norm_w[:, None]),
            "knw": np.ascontiguousarray(k_norm_w[:, None]),
            "iden": np.eye(128, dtype=MM_NP),
            "ones": np.ones((128, 1), dtype=MM_NP),
        })
    return in_maps


def _run(inputs, trace=False):
    if "nc" not in _cache:
        _cache["nc"] = _build()
    nc = _cache["nc"]
    in_maps = _prep_inputs(**inputs)
    res = run_bass_kernel_spmd(nc, in_maps, core_ids=list(range(DP * TP)),
                               trace=trace)
    out = np.zeros((B, S, HID), dtype=np.float32)
    for c in range(DP * TP):
        out[c // TP] += res.results[c]["out"]
    return out, res


def kernel(**inputs):
    out, _ = _run(inputs, trace=False)
    return out


# revision 13
# speedup vs baseline: 1.1455x; 1.0109x over previous
"""Trainium2 Bass kernel for nn_DFlashAttention_43774306681111.

Full-attention transformer block: QKV projection + per-head RMSNorm + neox
RoPE + GQA softmax attention (non-causal) + output projection.

Sharding (8 cores): 2-way data parallel over batch x 4-way tensor parallel
over heads. Core c handles batch c//4 and head group c%4 (q heads
4g..4g+3, kv head g). Each core computes a partial output [S, HID]
(its heads' contribution through Wo); the host sums the 4 partials per
batch. No device collectives.

Device layout: activations are kept transposed ([dim, token], dim on
partitions) so every matmul contracts on the partition axis:
  Q^T = Wq_tile^T @ X^T          (stationary Wq tile, moving X^T tile)
  S^T[k,q] = K^T_tile^T @ Q^T    (contraction d=128, one matmul per tile)
  softmax over k (= partitions): exp on ACT, sums via ones-vector matmul
  ctx^T[d,q] = V_tile^T @ expS^T (V stationary [k_tok, d])
  out[tok,hid] = ctxT_tile^T @ Wo
Matmuls run in float32r (fp32 storage, reduced-precision multiply);
PSUM accumulation is fp32. RoPE pairs (i, i+64) live on different
partitions, so the half-swap is done with two SBUF->SBUF DMAs and the
rotation sign is baked into the host-built sin table.

The attention loop is qb-major and the Wo matmuls for a finished query
block are deferred into the middle of the next block's QK stream, so the
PE never waits on a phase barrier.
"""
import numpy as np
from contextlib import ExitStack

import concourse.bass as bass
import concourse.tile as tile
from concourse import bacc, mybir
from concourse.bass_utils import run_bass_kernel_spmd

B, S, HID = 2, 2048, 2048
NH, NKV, D = 16, 4, 128
EPS = 1e-6
THETA = 1000000.0
SCALE = D ** -0.5

TP = 4                 # tensor-parallel groups (heads)
DP = 2                 # data-parallel over batch
HG = NH // TP          # q heads per core = 4
DQ = HG * D            # 512 q-proj cols per core
HALF = D // 2          # 64

F32 = mybir.dt.float32
F32R = mybir.dt.float32r
BF16 = mybir.dt.bfloat16

MM_DT = F32R           # matmul operand dtype (F32R or BF16)
MM_NP = np.float32     # host dtype fed into MM_DT dram tensors

HT = HID // 128        # 16 hid tiles
TBS = 512              # token block size
NTB = S // TBS         # 4 token blocks
KT = S // 128          # 16 key tiles
QB = S // TBS          # 4 query blocks
NDT = HG + 2           # 6 projection outputs: q0..q3, k, v^T

STAGGER = 3            # AV matmul emission lag behind QK/exp
WO_FLUSH_KT = 4        # emit deferred Wo matmuls after this many QK tiles

_cache = {}


def _build():
    nc = bacc.Bacc(None, target_bir_lowering=False, debug=False)

    xt = nc.dram_tensor("xt", [HID, S], MM_DT, kind="ExternalInput")
    wq = nc.dram_tensor("wq", [HID, DQ], MM_DT, kind="ExternalInput")
    wk = nc.dram_tensor("wk", [HID, D], MM_DT, kind="ExternalInput")
    wv = nc.dram_tensor("wv", [HID, D], MM_DT, kind="ExternalInput")
    wo = nc.dram_tensor("wo", [DQ, HID], MM_DT, kind="ExternalInput")
    cos2 = nc.dram_tensor("cos2", [D, S], F32, kind="ExternalInput")
    sin2 = nc.dram_tensor("sin2", [D, S], F32, kind="ExternalInput")
    qnw = nc.dram_tensor("qnw", [D, 1], F32, kind="ExternalInput")
    knw = nc.dram_tensor("knw", [D, 1], F32, kind="ExternalInput")
    iden_d = nc.dram_tensor("iden", [128, 128], MM_DT, kind="ExternalInput")
    ones_d = nc.dram_tensor("ones", [128, 1], MM_DT, kind="ExternalInput")
    out = nc.dram_tensor("out", [S, HID], F32, kind="ExternalOutput")

    with tile.TileContext(nc) as tc, ExitStack() as ctx:
        const = ctx.enter_context(tc.tile_pool(name="const", bufs=1))
        big = ctx.enter_context(tc.tile_pool(name="big", bufs=1))
        blk = ctx.enter_context(tc.tile_pool(name="blk", bufs=5))
        outp = ctx.enter_context(tc.tile_pool(name="outp", bufs=3))
        scratch = ctx.enter_context(tc.tile_pool(name="scratch", bufs=2))
        rows = ctx.enter_context(tc.tile_pool(name="rows", bufs=2))
        psum = ctx.enter_context(tc.tile_pool(name="psum", bufs=1, space="PSUM"))

        # ---- constants ----
        ident = const.tile([128, 128], MM_DT)
        nc.sync.dma_start(out=ident[:], in_=iden_d[:])
        ones_col = const.tile([128, 1], MM_DT)
        nc.sync.dma_start(out=ones_col[:], in_=ones_d[:])
        eps_row = const.tile([1, 1], F32)
        nc.vector.memset(eps_row, EPS)
        qnw_sb = const.tile([D, 1], F32)
        nc.sync.dma_start(out=qnw_sb[:], in_=qnw[:])
        knw_sb = const.tile([D, 1], F32)
        nc.sync.dma_start(out=knw_sb[:], in_=knw[:])

        # ---- resident weights / big activations (tag-shared slots) ----
        # weight DMAs are emitted inside the tb==0 loop, interleaved per hid
        # tile with the xt loads, so the first matmul starts ~2us in instead
        # of waiting for all weights.
        wq_sb = big.tile([128, HT, DQ], MM_DT, tag="bigw")
        wk_sb = big.tile([128, HT, D], MM_DT, tag="wk")
        wv_sb = big.tile([128, HT, D], MM_DT, tag="wv")
        cos_sb = big.tile([D, S], F32, tag="cos")
        sin_sb = big.tile([D, S], F32, tag="sin")

        qT = big.tile([D, HG, S], MM_DT, tag="qT")       # Q^T per head
        kT = big.tile([D, S], MM_DT, tag="kT")           # K^T
        vT = big.tile([D, S], MM_DT, tag="vT")           # V^T (pre-transpose)
        v_sb = big.tile([128, KT, D], MM_DT, tag="v")    # V [tok, d] tiles

        def stationary(ht, dt):
            if dt < HG:
                return wq_sb[:, ht, dt * D:(dt + 1) * D]
            if dt == HG:
                return wk_sb[:, ht, :]
            return wv_sb[:, ht, :]

        # ---- phase A: projections + rmsnorm + rope ----
        # eviction tails (rmsnorm + rope) are deferred into the next token
        # block's matmul stream so their ssq matmuls never stall the PE
        pending_evict = []

        def flush_evict():
            while pending_evict:
                pending_evict.pop(0)()

        for tb in range(NTB):
            tsl = slice(tb * TBS, (tb + 1) * TBS)
            accs = [psum.tile([128, TBS], F32, tag=f"p{'ABCDEF'[dt]}",
                              name=f"acc_{tb}_{dt}") for dt in range(NDT)]
            for ht in range(HT):
                if tb == 0:
                    hsl = slice(ht * 128, (ht + 1) * 128)
                    nc.sync.dma_start(out=wq_sb[:, ht, :], in_=wq[hsl, :])
                    nc.sync.dma_start(out=wk_sb[:, ht, :], in_=wk[hsl, :])
                    nc.sync.dma_start(out=wv_sb[:, ht, :], in_=wv[hsl, :])
                if ht == 1:
                    # rope tables arrive ahead of this tb's (deferred) eviction
                    nc.sync.dma_start(out=cos_sb[:, tsl], in_=cos2[:, tsl])
                    nc.sync.dma_start(out=sin_sb[:, tsl], in_=sin2[:, tsl])
                if ht == 2:
                    flush_evict()  # previous tb's eviction tail rides this stream
                xt_t = blk.tile([128, TBS], MM_DT, tag="blk", name=f"xt_{tb}_{ht}")
                nc.sync.dma_start(out=xt_t[:], in_=xt[ht * 128:(ht + 1) * 128, tsl])
                for dt in range(NDT):
                    nc.tensor.matmul(accs[dt][:], stationary(ht, dt), xt_t[:],
                                     start=(ht == 0), stop=(ht == HT - 1))
            for dt in range(NDT):
                acc = accs[dt]
                if dt == NDT - 1:
                    nc.scalar.copy(vT[:, tsl], acc[:])
                    continue
                w_ap = qnw_sb if dt < HG else knw_sb
                # single psum read (alternating engines) frees the bank fast
                raw = scratch.tile([128, TBS], F32, tag="raw", bufs=2,
                                   name=f"raw_{tb}_{dt}")
                if dt % 2 == 0:
                    nc.vector.tensor_copy(raw[:], acc[:])
                else:
                    nc.scalar.copy(raw[:], acc[:])
                # qn/q2 computed eagerly so the deferred ssq matmul never
                # stalls the PE stream it is later emitted into
                qn = scratch.tile([128, TBS], F32, tag="qn", bufs=6,
                                  name=f"qn_{tb}_{dt}")
                nc.scalar.activation(qn[:], raw[:],
                                     mybir.ActivationFunctionType.Copy,
                                     scale=w_ap[:])
                q2 = scratch.tile([128, TBS], MM_DT, tag="q2", bufs=6,
                                  name=f"q2_{tb}_{dt}")
                nc.vector.tensor_mul(q2[:], raw[:], raw[:])

                def evict_tail(tb=tb, dt=dt, qn=qn, q2=q2, tsl=tsl):
                    ssq = psum.tile([1, TBS], F32, tag="small", bufs=1,
                                    name=f"ssq_{tb}_{dt}")
                    nc.tensor.matmul(ssq[:], ones_col[:], q2[:],
                                     start=True, stop=True)
                    rstd = rows.tile([1, TBS], F32, tag="rstd", bufs=2,
                                     name=f"rstd_{tb}_{dt}")
                    nc.scalar.activation(rstd[:], ssq[:],
                                         mybir.ActivationFunctionType.Sqrt,
                                         scale=1.0 / D, bias=eps_row[:])
                    nc.vector.reciprocal(rstd[:], rstd[:])
                    rstdb = scratch.tile([128, TBS], F32, tag="bcast", bufs=3,
                                         name=f"rstdb_{tb}_{dt}")
                    nc.gpsimd.partition_broadcast(rstdb[:], rstd[:])
                    # rope: swap halves via SBUF->SBUF DMA; sign baked in sin2
                    xsw = scratch.tile([128, TBS], F32, tag="xsw", bufs=3,
                                       name=f"xsw_{tb}_{dt}")
                    nc.sync.dma_start(out=xsw[0:HALF, :], in_=qn[HALF:D, :])
                    nc.sync.dma_start(out=xsw[HALF:D, :], in_=qn[0:HALF, :])
                    tmp = scratch.tile([128, TBS], F32, tag="tmp", bufs=2,
                                       name=f"tmp_{tb}_{dt}")
                    nc.vector.tensor_mul(tmp[:], qn[:], cos_sb[:, tsl])
                    sv = scratch.tile([128, TBS], F32, tag="sv", bufs=2,
                                      name=f"sv_{tb}_{dt}")
                    nc.vector.tensor_mul(sv[:], xsw[:], sin_sb[:, tsl])
                    qro = scratch.tile([128, TBS], F32, tag="qro", bufs=2,
                                       name=f"qro_{tb}_{dt}")
                    nc.vector.tensor_add(qro[:], tmp[:], sv[:])
                    dest = qT[:, dt, tsl] if dt < HG else kT[:, tsl]
                    nc.vector.tensor_mul(dest, qro[:], rstdb[:])
                pending_evict.append(evict_tail)

        # ---- V^T -> V transpose (PE) ----
        for kt in range(KT):
            tp = psum.tile([128, 128], MM_DT, tag="pE", name=f"tp_{kt}")
            nc.tensor.transpose(tp[:], vT[:, kt * 128:(kt + 1) * 128], ident[:])
            nc.vector.tensor_copy(v_sb[:, kt, :], tp[:])
            if kt == 2:
                flush_evict()  # last tb's eviction tail rides the transposes

        # ctx^T per head; slots reuse cos/sin/vT space (all dead by phase B)
        ctxT = [big.tile([D, S], MM_DT, tag=t, name=f"ctxT_{h}")
                for h, t in enumerate(["cos", "sin", "vT", "ctx3"])]

        # wo loads overlap the first attention blocks ("bigw" frees after
        # the last projection matmul)
        wo_sb = big.tile([128, HG, HID], MM_DT, tag="bigw")
        for ct in range(HG):
            nc.sync.dma_start(out=wo_sb[:, ct, :],
                              in_=wo[ct * 128:(ct + 1) * 128, :])

        # ---- phase B: attention, qb-major, with Wo folded in ----
        pending_wo = []

        def emit_wo(qb):
            qb_out = []
            for tt in range(qb * (TBS // 128), (qb + 1) * (TBS // 128)):
                for hc in range(HID // TBS):
                    def thunk(tt=tt, hc=hc):
                        o_ps = psum.tile([128, TBS], F32,
                                         tag=f"p{'EF'[(tt * 4 + hc) % 2]}",
                                         name=f"o_{tt}_{hc}")
                        for ct in range(HG):
                            nc.tensor.matmul(
                                o_ps[:],
                                ctxT[ct][:, tt * 128:(tt + 1) * 128],
                                wo_sb[:, ct, hc * TBS:(hc + 1) * TBS],
                                start=(ct == 0), stop=(ct == HG - 1))
                        o_sb = outp.tile([128, TBS], F32, tag="osb",
                                         name=f"osb_{tt}_{hc}")
                        nc.scalar.copy(o_sb[:], o_ps[:])
                        nc.sync.dma_start(
                            out=out[tt * 128:(tt + 1) * 128,
                                    hc * TBS:(hc + 1) * TBS],
                            in_=o_sb[:])
                    qb_out.append(thunk)
            return qb_out

        def flush_wo():
            while pending_wo:
                pending_wo.pop(0)()

        for qb in range(QB):
            qsl = slice(qb * TBS, (qb + 1) * TBS)
            for h in range(HG):
                blk_i = qb * HG + h
                ctx_ps = psum.tile([128, TBS], F32,
                                   tag=f"p{'CD'[blk_i % 2]}",
                                   name=f"ctx_{h}_{qb}")
                sum_ps = psum.tile([1, TBS], F32, tag="small", bufs=1,
                                   name=f"sum_{h}_{qb}")
                pend = []

                def flush_one():
                    kt0, e0 = pend.pop(0)
                    nc.tensor.matmul(ctx_ps[:], v_sb[:, kt0, :], e0[:],
                                     start=(kt0 == 0), stop=(kt0 == KT - 1))
                    nc.tensor.matmul(sum_ps[:], ones_col[:], e0[:],
                                     start=(kt0 == 0), stop=(kt0 == KT - 1))

                for kt in range(KT):
                    s_ps = psum.tile([128, TBS], F32,
                                     tag=f"p{'ABG'[(blk_i * KT + kt) % 3]}",
                                     name=f"s_{h}_{qb}_{kt}")
                    nc.tensor.matmul(s_ps[:], kT[:, kt * 128:(kt + 1) * 128],
                                     qT[:, h, qsl], start=True, stop=True)
                    e = blk.tile([128, TBS], MM_DT, tag="blk",
                                 name=f"e_{h}_{qb}_{kt}")
                    nc.scalar.activation(e[:], s_ps[:],
                                         mybir.ActivationFunctionType.Exp,
                                         scale=SCALE)
                    pend.append((kt, e))
                    if len(pend) > STAGGER:
                        flush_one()
                    if h == 0 and kt == WO_FLUSH_KT:
                        flush_wo()  # previous qb's Wo rides in the QK stream
                while pend:
                    flush_one()

                recip = rows.tile([1, TBS], F32, tag="recip",
                                  name=f"recip_{h}_{qb}")
                nc.vector.reciprocal(recip[:], sum_ps[:])
                recipb = scratch.tile([128, TBS], F32, tag="bcast", bufs=3,
                                      name=f"recipb_{h}_{qb}")
                nc.gpsimd.partition_broadcast(recipb[:], recip[:])
                nc.vector.tensor_mul(ctxT[h][:, qsl], ctx_ps[:], recipb[:])
            pending_wo = emit_wo(qb)
        flush_wo()

    nc.compile()
    return nc


def _prep_inputs(hidden_states, positions, Wq, Wk, Wv, Wo, q_norm_w, k_norm_w):
    hidden_states = np.asarray(hidden_states, dtype=np.float32)
    positions = np.asarray(positions)
    Wq = np.asarray(Wq, dtype=np.float32)
    Wk = np.asarray(Wk, dtype=np.float32)
    Wv = np.asarray(Wv, dtype=np.float32)
    Wo = np.asarray(Wo, dtype=np.float32)
    q_norm_w = np.asarray(q_norm_w, dtype=np.float32)
    k_norm_w = np.asarray(k_norm_w, dtype=np.float32)

    inv_freq = THETA ** (-np.arange(HALF, dtype=np.float32) / HALF)
    in_maps = []
    for c in range(DP * TP):
        b, g = divmod(c, TP)
        freqs = positions[b].astype(np.float32)[:, None] * inv_freq[None, :]  # [S,64]
        cos = np.cos(freqs).T.astype(np.float32)      # [64, S]
        sin = np.sin(freqs).T.astype(np.float32)
        cos2 = np.ascontiguousarray(np.concatenate([cos, cos], axis=0))   # [128,S]
        sin2 = np.ascontiguousarray(np.concatenate([-sin, sin], axis=0))  # [128,S]
        in_maps.append({
            "xt": np.ascontiguousarray(hidden_states[b].T).astype(MM_NP),
            "wq": np.ascontiguousarray(Wq[:, g * DQ:(g + 1) * DQ]).astype(MM_NP),
            "wk": np.ascontiguousarray(Wk[:, g * D:(g + 1) * D]).astype(MM_NP),
            "wv": np.ascontiguousarray(Wv[:, g * D:(g + 1) * D]).astype(MM_NP),
            "wo": np.ascontiguousarray(Wo[g * DQ:(g + 1) * DQ, :]).astype(MM_NP),
            "cos2": cos2,
            "sin2": sin2,
            "qnw": np.ascontiguousarray(q_norm_w[:, None]),
            "knw": np.ascontiguousarray(k_norm_w[:, None]),
            "iden": np.eye(128, dtype=MM_NP),
            "ones": np.ones((128, 1), dtype=MM_NP),
        })
    return in_maps


def _run(inputs, trace=False):
    if "nc" not in _cache:
        _cache["nc"] = _build()
    nc = _cache["nc"]
    in_maps = _prep_inputs(**inputs)
    res = run_bass_kernel_spmd(nc, in_maps, core_ids=list(range(DP * TP)),
                               trace=trace)
    out = np.zeros((B, S, HID), dtype=np.float32)
    for c in range(DP * TP):
        out[c // TP] += res.results[c]["out"]
    return out, res


def kernel(**inputs):
    out, _ = _run(inputs, trace=False)
    return out
